# revision 7
# baseline (speedup 1.0000x reference)
"""AngleEmbedding kernel for 8 TRN2 NeuronCores.

The reference applies, per qubit q, the overwrite-semantics "rotation"
    new[i0] = 1j*sin(th/2)*state[i1];  new[i1] = cos(th/2)*state[i1]
(i1 = index with bit q set). Both outputs depend only on the bit=1
amplitudes. The initial state |0...0> has zero amplitude at every index
with any bit set, so the state is identically zero after the first
rotation and stays zero: the exact output is zeros((8, 2^20), complex64)
for every input x.

The kernel therefore reduces to materializing the 64 MiB zero output at
HBM write bandwidth. Sharding (per the state-vector-parallel hint): the
2^20 state axis is split across the 8 cores; each core owns 2^17 states
per batch row = 8 MiB of f32 (re,im) pairs, written by large HWDGE DMAs.

Per-core schedule (all zero-fill DMAs read one SBUF tile, so the write
stream never re-reads HBM):
  - gpsimd memsets a tiny [128, 256] head of the [128, 2048] zero tile
    (emitted pre-Block, so it issues right after NEFF init ~7.3us);
  - vector memsets the tile tail [256:2048] in parallel;
  - sync launches a 0.5 MiB head chunk from the tile head via a step-0
    repeat source AP (first HBM packets ~9.1us), then 3x 1 MiB bulk
    chunks with 8 KiB source runs; scalar (the second HWDGE ring)
    launches 4x 1 MiB bulk chunks plus a 0.5 MiB tail chunk. The 16
    SDMA engines interleave both rings at packet granularity and stream
    the 8 MiB gap-free at ~405-415 GB/s per core (fabric ceiling 435;
    two cores share each 716 GB/s HBM stack, so all-8-core sustained is
    HBM-arbitration bound).
Measured: ~31.4 us best, ~33-36 us under ambient HBM contention; the
fixed NEFF init (~7.3 us to first user instruction) and tail (last-byte
write receipt + exit barrier, ~2.5 us) bound further improvement.
"""

import numpy as np

N_CORES = 8
BATCH = 8
N_QUBITS = 20
STATES = 1 << N_QUBITS                      # 1048576
SHARD_STATES = STATES // N_CORES            # 131072 states per core
SHARD_F32 = BATCH * SHARD_STATES * 2        # 2097152 f32 per core (8 MiB)
OUT_P = 128
OUT_F = SHARD_F32 // OUT_P                  # 16384
TILE_F = 2048                               # zero tile: [128, 2048] f32 = 1 MiB
M0_F = 256                                  # early head: [128, 256] f32

_CACHE = {}


def _build_nc():
    import concourse.bass as bass
    import concourse.mybir as mybir

    nc = bass.Bass()
    x = nc.declare_dram_parameter(
        "x", [BATCH, N_QUBITS], mybir.dt.float32, isOutput=False
    )
    out = nc.declare_dram_parameter(
        "out", [OUT_P, OUT_F], mybir.dt.float32, isOutput=True
    )

    half_f = TILE_F // 2                    # 1024: head/tail chunk width
    rep0 = half_f // M0_F
    n_bulk = (OUT_F - TILE_F) // TILE_F     # 7x 1 MiB bulk chunks
    n_dmas = n_bulk + 2

    def rep_ap(t, rep):
        # Read the tile `rep` times: partition dim first (must have nonzero
        # step), then a step-0 repeat dim over the per-partition run.
        return bass.AP(t.tensor, t.offset, [list(t.ap[0]), [0, rep], list(t.ap[1])])

    with (
        nc.sbuf_tensor([OUT_P, TILE_F], mybir.dt.float32) as ztile,
        nc.sbuf_tensor([BATCH, N_QUBITS], mybir.dt.float32) as xtile,
        nc.semaphore() as s0,
        nc.semaphore() as s1,
        nc.semaphore() as dsem,
        nc.semaphore() as xsem,
    ):
        t = ztile[:]
        t0 = ztile[:, :M0_F]
        th = ztile[:, :half_f]
        # Emitted before the Block: these land right after NEFF init.
        nc.gpsimd.memset(t0, 0.0).then_inc(s0, 1)
        nc.vector.memset(ztile[:, M0_F:], 0.0).then_inc(s1, 1)

        # Layout: [0:1024] head chunk, 7x [.. +2048] bulk, [15360:] tail.
        def bulk_dst(k):
            return out[:, half_f + k * TILE_F: half_f + (k + 1) * TILE_F]

        # Raw engine streams (no Block): skips the exit drain + all-engine
        # semaphore butterfly (~0.9 us). Every DMA is sem-waited before its
        # guarding engine retires: gpsimd waits the x-load, sync waits all
        # n_dmas zero-fill completions (covering scalar's too), so the NEFF
        # cannot complete before the last byte's HBM write receipt.

        # Consume the angle input (the output is independent of it).
        nc.gpsimd.dma_start(out=xtile[:], in_=x[:]).then_inc(xsem, 16)
        nc.gpsimd.wait_ge(xsem, 16)

        # Bulk + tail read tile regions covered by both memsets.
        nc.scalar.wait_ge(s0, 1)
        nc.scalar.wait_ge(s1, 1)
        for k in range(n_bulk // 2, n_bulk):
            nc.scalar.dma_start(out=bulk_dst(k), in_=t).then_inc(dsem, 16)
        nc.scalar.dma_start(
            out=out[:, OUT_F - half_f:], in_=th
        ).then_inc(dsem, 16)

        nc.sync.wait_ge(s0, 1)
        nc.sync.dma_start(
            out=out[:, :half_f].rearrange("p (r f) -> p r f", r=rep0),
            in_=rep_ap(t0, rep0),
        ).then_inc(dsem, 16)
        nc.sync.wait_ge(s1, 1)
        for k in range(n_bulk // 2):
            nc.sync.dma_start(out=bulk_dst(k), in_=t).then_inc(dsem, 16)
        nc.sync.wait_ge(dsem, 16 * n_dmas)

    return nc


def _run(x, trace=False):
    from concourse.bass_utils import run_bass_kernel_spmd

    if "nc" not in _CACHE:
        _CACHE["nc"] = _build_nc()
    nc = _CACHE["nc"]

    xf = np.ascontiguousarray(np.asarray(x, dtype=np.float32))
    assert xf.shape == (BATCH, N_QUBITS)
    in_maps = [{"x": xf} for _ in range(N_CORES)]
    try:
        res = run_bass_kernel_spmd(
            nc, in_maps, core_ids=list(range(N_CORES)), trace=trace
        )
    except Exception:
        # The axon-tunneled device occasionally throws a transient
        # NRT_EXEC_UNIT_UNRECOVERABLE; one retry clears it.
        res = run_bass_kernel_spmd(
            nc, in_maps, core_ids=list(range(N_CORES)), trace=trace
        )
    # Core i holds states [i*SHARD_STATES, (i+1)*SHARD_STATES) for each
    # batch row, as interleaved (re, im) f32 pairs.
    parts = [
        res.results[i]["out"].reshape(BATCH, SHARD_STATES * 2)
        for i in range(N_CORES)
    ]
    full = np.ascontiguousarray(np.concatenate(parts, axis=1))
    return full.view(np.complex64), res


def kernel(x):
    out, _ = _run(x, trace=False)
    return out


# revision 9
# speedup vs baseline: 1.0862x; 1.0862x over previous
"""AngleEmbedding kernel for 8 TRN2 NeuronCores.

The reference applies, per qubit q, the overwrite-semantics "rotation"
    new[i0] = 1j*sin(th/2)*state[i1];  new[i1] = cos(th/2)*state[i1]
(i1 = index with bit q set). Both outputs depend only on the bit=1
amplitudes. The initial state |0...0> has zero amplitude at every index
with any bit set, so the state is identically zero after the first
rotation and stays zero: the exact output is zeros((8, 2^20), complex64)
for every input x.

The kernel therefore reduces to materializing the 64 MiB zero output at
HBM write bandwidth. Sharding (per the state-vector-parallel hint): the
2^20 state axis is split across the 8 cores; each core owns 2^17 states
per batch row = 8 MiB of f32 (re,im) pairs, written by large HWDGE DMAs.

Per-core schedule (all zero-fill DMAs read one SBUF tile, so the write
stream never re-reads HBM):
  - gpsimd memsets a tiny [128, 256] head of the [128, 2048] zero tile
    (emitted pre-Block, so it issues right after NEFF init ~7.3us);
  - vector memsets the tile tail [256:2048] in parallel;
  - sync launches a 0.5 MiB head chunk from the tile head via a step-0
    repeat source AP (first HBM packets ~9.1us), then 3x 1 MiB bulk
    chunks with 8 KiB source runs; scalar (the second HWDGE ring)
    launches 4x 1 MiB bulk chunks plus a 0.5 MiB tail chunk. The 16
    SDMA engines interleave both rings at packet granularity and stream
    the 8 MiB gap-free at ~405-415 GB/s per core (fabric ceiling 435;
    two cores share each 716 GB/s HBM stack, so all-8-core sustained is
    HBM-arbitration bound).
Measured: ~31.4 us best, ~33-36 us under ambient HBM contention; the
fixed NEFF init (~7.3 us to first user instruction) and tail (last-byte
write receipt + exit barrier, ~2.5 us) bound further improvement.
"""

import numpy as np

N_CORES = 8
BATCH = 8
N_QUBITS = 20
STATES = 1 << N_QUBITS                      # 1048576
SHARD_STATES = STATES // N_CORES            # 131072 states per core
SHARD_F32 = BATCH * SHARD_STATES * 2        # 2097152 f32 per core (8 MiB)
OUT_P = 128
OUT_F = SHARD_F32 // OUT_P                  # 16384
TILE_F = 2048                               # zero tile: [128, 2048] f32 = 1 MiB
M0_F = 256                                  # early head: [128, 256] f32

_CACHE = {}


def _build_nc():
    import concourse.bass as bass
    import concourse.mybir as mybir

    # Skip Bass.__init__'s trailing all_engine_barrier: it only orders the
    # built-in const-tile memsets against engines that read them, and this
    # kernel reads none. Without it, engines reach user code straight from
    # their preambles (~6.5-7.2 us instead of ~7.5), and every cross-engine
    # dependency here is explicitly semaphore-guarded.
    orig_barrier = bass.Bass.all_engine_barrier
    bass.Bass.all_engine_barrier = lambda self, *a, **k: None
    try:
        nc = bass.Bass()
    finally:
        bass.Bass.all_engine_barrier = orig_barrier
    x = nc.declare_dram_parameter(
        "x", [BATCH, N_QUBITS], mybir.dt.float32, isOutput=False
    )
    out = nc.declare_dram_parameter(
        "out", [OUT_P, OUT_F], mybir.dt.float32, isOutput=True
    )

    half_f = TILE_F // 2                    # 1024: head/tail chunk width
    rep0 = half_f // M0_F
    n_bulk = (OUT_F - TILE_F) // TILE_F     # 7x 1 MiB bulk chunks
    n_dmas = n_bulk + 2

    def rep_ap(t, rep):
        # Read the tile `rep` times: partition dim first (must have nonzero
        # step), then a step-0 repeat dim over the per-partition run.
        return bass.AP(t.tensor, t.offset, [list(t.ap[0]), [0, rep], list(t.ap[1])])

    with (
        nc.sbuf_tensor([OUT_P, TILE_F], mybir.dt.float32) as ztile,
        nc.sbuf_tensor([BATCH, N_QUBITS], mybir.dt.float32) as xtile,
        nc.semaphore() as s0,
        nc.semaphore() as s1,
        nc.semaphore() as dsem,
        nc.semaphore() as xsem,
    ):
        t = ztile[:]
        t0 = ztile[:, :M0_F]
        th = ztile[:, :half_f]
        # Both memset stages on vector: its preamble ends earliest (~6.5 us)
        # once the init barrier is gone, so s0 fires before sync is even
        # ready to dispatch, and s1 (~1 us later) unlocks the bulk with no
        # ring bubble after the head chunk.
        nc.vector.memset(t0, 0.0).then_inc(s0, 1)
        nc.vector.memset(ztile[:, M0_F:], 0.0).then_inc(s1, 1)

        # Layout: [0:1024] head chunk, 7x [.. +2048] bulk, [15360:] tail.
        def bulk_dst(k):
            return out[:, half_f + k * TILE_F: half_f + (k + 1) * TILE_F]

        # Raw engine streams (no Block): skips the exit drain + all-engine
        # semaphore butterfly (~0.9 us). Every DMA is sem-waited before its
        # guarding engine retires: gpsimd waits the x-load, sync waits all
        # n_dmas zero-fill completions (covering scalar's too), so the NEFF
        # cannot complete before the last byte's HBM write receipt.

        # Consume the angle input (the output is independent of it).
        nc.gpsimd.dma_start(out=xtile[:], in_=x[:]).then_inc(xsem, 16)
        nc.gpsimd.wait_ge(xsem, 16)

        # Bulk + tail read tile regions covered by both memsets.
        nc.scalar.wait_ge(s0, 1)
        nc.scalar.wait_ge(s1, 1)
        for k in range(n_bulk // 2, n_bulk):
            nc.scalar.dma_start(out=bulk_dst(k), in_=t).then_inc(dsem, 16)
        nc.scalar.dma_start(
            out=out[:, OUT_F - half_f:], in_=th
        ).then_inc(dsem, 16)

        nc.sync.wait_ge(s0, 1)
        nc.sync.dma_start(
            out=out[:, :half_f].rearrange("p (r f) -> p r f", r=rep0),
            in_=rep_ap(t0, rep0),
        ).then_inc(dsem, 16)
        nc.sync.wait_ge(s1, 1)
        for k in range(n_bulk // 2):
            nc.sync.dma_start(out=bulk_dst(k), in_=t).then_inc(dsem, 16)
        nc.sync.wait_ge(dsem, 16 * n_dmas)

    return nc


def _run(x, trace=False):
    from concourse.bass_utils import run_bass_kernel_spmd

    if "nc" not in _CACHE:
        _CACHE["nc"] = _build_nc()
    nc = _CACHE["nc"]

    xf = np.ascontiguousarray(np.asarray(x, dtype=np.float32))
    assert xf.shape == (BATCH, N_QUBITS)
    in_maps = [{"x": xf} for _ in range(N_CORES)]
    try:
        res = run_bass_kernel_spmd(
            nc, in_maps, core_ids=list(range(N_CORES)), trace=trace
        )
    except Exception:
        # The axon-tunneled device occasionally throws a transient
        # NRT_EXEC_UNIT_UNRECOVERABLE; one retry clears it.
        res = run_bass_kernel_spmd(
            nc, in_maps, core_ids=list(range(N_CORES)), trace=trace
        )
    # Core i holds states [i*SHARD_STATES, (i+1)*SHARD_STATES) for each
    # batch row, as interleaved (re, im) f32 pairs.
    parts = [
        res.results[i]["out"].reshape(BATCH, SHARD_STATES * 2)
        for i in range(N_CORES)
    ]
    full = np.ascontiguousarray(np.concatenate(parts, axis=1))
    return full.view(np.complex64), res


def kernel(x):
    out, _ = _run(x, trace=False)
    return out


# revision 11
# speedup vs baseline: 1.1052x; 1.0174x over previous
"""AngleEmbedding kernel for 8 TRN2 NeuronCores.

The reference applies, per qubit q, the overwrite-semantics "rotation"
    new[i0] = 1j*sin(th/2)*state[i1];  new[i1] = cos(th/2)*state[i1]
(i1 = index with bit q set). Both outputs depend only on the bit=1
amplitudes. The initial state |0...0> has zero amplitude at every index
with any bit set, so the state is identically zero after the first
rotation and stays zero: the exact output is zeros((8, 2^20), complex64)
for every input x.

The kernel therefore reduces to materializing the 64 MiB zero output at
HBM write bandwidth. Sharding (per the state-vector-parallel hint): the
2^20 state axis is split across the 8 cores; each core owns 2^17 states
per batch row = 8 MiB of f32 (re,im) pairs, written by large HWDGE DMAs.

Per-core schedule (all zero-fill DMAs read one SBUF tile, so the write
stream never re-reads HBM):
  - gpsimd memsets a tiny [128, 256] head of the [128, 2048] zero tile
    (emitted pre-Block, so it issues right after NEFF init ~7.3us);
  - vector memsets the tile tail [256:2048] in parallel;
  - sync launches a 0.5 MiB head chunk from the tile head via a step-0
    repeat source AP (first HBM packets ~9.1us), then 3x 1 MiB bulk
    chunks with 8 KiB source runs; scalar (the second HWDGE ring)
    launches 4x 1 MiB bulk chunks plus a 0.5 MiB tail chunk. The 16
    SDMA engines interleave both rings at packet granularity and stream
    the 8 MiB gap-free at ~405-415 GB/s per core (fabric ceiling 435;
    two cores share each 716 GB/s HBM stack, so all-8-core sustained is
    HBM-arbitration bound).
Measured: ~31.4 us best, ~33-36 us under ambient HBM contention; the
fixed NEFF init (~7.3 us to first user instruction) and tail (last-byte
write receipt + exit barrier, ~2.5 us) bound further improvement.
"""

import numpy as np

N_CORES = 8
BATCH = 8
N_QUBITS = 20
STATES = 1 << N_QUBITS                      # 1048576
SHARD_STATES = STATES // N_CORES            # 131072 states per core
SHARD_F32 = BATCH * SHARD_STATES * 2        # 2097152 f32 per core (8 MiB)
OUT_P = 128
OUT_F = SHARD_F32 // OUT_P                  # 16384
TILE_F = 2048                               # zero tile: [128, 2048] f32 = 1 MiB
M0_F = 256                                  # early head: [128, 256] f32

_CACHE = {}


def _build_nc():
    import concourse.bass as bass
    import concourse.mybir as mybir

    # Skip Bass.__init__'s trailing all_engine_barrier: it only orders the
    # built-in const-tile memsets against engines that read them, and this
    # kernel reads none. Without it, engines reach user code straight from
    # their preambles (~6.5-7.2 us instead of ~7.5), and every cross-engine
    # dependency here is explicitly semaphore-guarded.
    orig_barrier = bass.Bass.all_engine_barrier
    bass.Bass.all_engine_barrier = lambda self, *a, **k: None
    try:
        nc = bass.Bass()
    finally:
        bass.Bass.all_engine_barrier = orig_barrier
    x = nc.declare_dram_parameter(
        "x", [BATCH, N_QUBITS], mybir.dt.float32, isOutput=False
    )
    out = nc.declare_dram_parameter(
        "out", [OUT_P, OUT_F], mybir.dt.float32, isOutput=True
    )

    half_f = TILE_F // 2                    # 1024: head/tail chunk width
    rep0 = half_f // M0_F
    n_bulk = (OUT_F - TILE_F) // TILE_F     # 7x 1 MiB bulk chunks
    n_dmas = n_bulk + 2

    def rep_ap(t, rep):
        # Read the tile `rep` times: partition dim first (must have nonzero
        # step), then a step-0 repeat dim over the per-partition run.
        return bass.AP(t.tensor, t.offset, [list(t.ap[0]), [0, rep], list(t.ap[1])])

    m1a_end = 1280                          # gpsimd/vector split of the tail

    with (
        nc.sbuf_tensor([OUT_P, TILE_F], mybir.dt.float32) as ztile,
        nc.sbuf_tensor([BATCH, N_QUBITS], mybir.dt.float32) as xtile,
        nc.semaphore() as s0,
        nc.semaphore() as sa,
        nc.semaphore() as sb,
        nc.semaphore() as dsem,
        nc.semaphore() as xsem,
    ):
        t = ztile[:]
        t0 = ztile[:, :M0_F]
        th = ztile[:, :half_f]
        # m0 on vector (earliest preamble, ~6.5 us, unlocks the head chunk
        # by ~7.4). The tile tail is split: vector pays a ~0.7 us inter-op
        # gap before its second memset, so gpsimd (free after the const
        # memsets ~7.2 us, slower per byte) takes [M0_F:1280] while vector
        # takes [1280:]; both finish ~8.7 us, ~0.9 us before a single
        # vector memset would, so the bulk feeds the rings with no gap
        # after the head chunk drains.
        nc.vector.memset(t0, 0.0).then_inc(s0, 1)
        nc.vector.memset(ztile[:, m1a_end:], 0.0).then_inc(sb, 1)
        nc.gpsimd.memset(ztile[:, M0_F:m1a_end], 0.0).then_inc(sa, 1)

        # Layout: [0:1024] head chunk, 7x [.. +2048] bulk, [15360:] tail.
        def bulk_dst(k):
            return out[:, half_f + k * TILE_F: half_f + (k + 1) * TILE_F]

        # Raw engine streams (no Block): skips the exit drain + all-engine
        # semaphore butterfly (~0.9 us). Every DMA is sem-waited before its
        # guarding engine retires: gpsimd waits the x-load, sync waits all
        # n_dmas zero-fill completions (covering scalar's too), so the NEFF
        # cannot complete before the last byte's HBM write receipt.

        # Consume the angle input (the output is independent of it).
        nc.gpsimd.dma_start(out=xtile[:], in_=x[:]).then_inc(xsem, 16)
        nc.gpsimd.wait_ge(xsem, 16)

        # scalar: tail chunk first (reads [0:1024] = m0+m1a only, so it can
        # launch before vector's half finishes), then bulk (needs the full
        # tile: all three memset sems).
        nc.scalar.wait_ge(s0, 1)
        nc.scalar.wait_ge(sa, 1)
        nc.scalar.dma_start(
            out=out[:, OUT_F - half_f:], in_=th
        ).then_inc(dsem, 16)
        nc.scalar.wait_ge(sb, 1)
        for k in range(n_bulk // 2, n_bulk):
            nc.scalar.dma_start(out=bulk_dst(k), in_=t).then_inc(dsem, 16)

        nc.sync.wait_ge(s0, 1)
        nc.sync.dma_start(
            out=out[:, :half_f].rearrange("p (r f) -> p r f", r=rep0),
            in_=rep_ap(t0, rep0),
        ).then_inc(dsem, 16)
        nc.sync.wait_ge(sa, 1)
        nc.sync.wait_ge(sb, 1)
        for k in range(n_bulk // 2):
            nc.sync.dma_start(out=bulk_dst(k), in_=t).then_inc(dsem, 16)
        nc.sync.wait_ge(dsem, 16 * n_dmas)

    return nc


def _run(x, trace=False):
    from concourse.bass_utils import run_bass_kernel_spmd

    if "nc" not in _CACHE:
        _CACHE["nc"] = _build_nc()
    nc = _CACHE["nc"]

    xf = np.ascontiguousarray(np.asarray(x, dtype=np.float32))
    assert xf.shape == (BATCH, N_QUBITS)
    in_maps = [{"x": xf} for _ in range(N_CORES)]
    try:
        res = run_bass_kernel_spmd(
            nc, in_maps, core_ids=list(range(N_CORES)), trace=trace
        )
    except Exception:
        # The axon-tunneled device occasionally throws a transient
        # NRT_EXEC_UNIT_UNRECOVERABLE; one retry clears it.
        res = run_bass_kernel_spmd(
            nc, in_maps, core_ids=list(range(N_CORES)), trace=trace
        )
    # Core i holds states [i*SHARD_STATES, (i+1)*SHARD_STATES) for each
    # batch row, as interleaved (re, im) f32 pairs.
    parts = [
        res.results[i]["out"].reshape(BATCH, SHARD_STATES * 2)
        for i in range(N_CORES)
    ]
    full = np.ascontiguousarray(np.concatenate(parts, axis=1))
    return full.view(np.complex64), res


def kernel(x):
    out, _ = _run(x, trace=False)
    return out
